# revision 12
# baseline (speedup 1.0000x reference)
"""Trainium2 Bass kernel for the MAB-style dense transformer block.

Reference computation (per batch b of 4, channel-major [D=512, S=2048]):
    q = Wq @ Q + bq                  # [D, Sq]
    k = Wk @ K + bk                  # [D, Sk]
    v = Wv @ K + bv                  # [D, Sk]
    per head h (8 heads x 64 ch):
      logits = (q_h * 0.125)^T k_h   # [Sq, Sk]
      w = softmax(logits, axis=-1)
      att_h = (w @ v_h^T)^T          # [64, Sq]
    x  = q + att                     # residual
    y  = LN_ch(x; g0, b0)            # layernorm over channels
    z  = y + relu(Wo @ y + bo)
    out = LN_ch(z; g1, b1)

Sharding: 8 cores = batch (4) x query-half (2). Each core handles
[D, 1024] of queries for one batch with the full K/V — zero
cross-core communication.

On-core dataflow (all matmuls in float32r = full-rate fp32):
  - q/k projections channel-major; v projection sequence-major
    [Sk, 64] per head with a ones-column appended so the attention
    AV matmul (M=65) also produces the softmax denominator row.
  - logitsT [k-part, q-free] per (head, k-block); ACT exp with
    scale=0.125 folded in; no max-subtraction (logits are O(5)).
  - attention output normalized by the denominator via DVE (reciprocal
    of the ones-column row, broadcast across partitions by a K=1
    matmul); heads assembled into channel-major blocks by SBUF->SBUF
    DMA (the only engine that can cross partition bases).
  - channel-axis layernorm stats via ones-column matmuls; per-q
    broadcasts via K=1 matmuls (A = g*rstd, B = -mean*rstd*g + b),
    so each LN costs ~3 DVE passes. Squares and SBUF-only residual
    adds run on GpSimd; relu on ScalarE; everything else elementwise
    on VectorE.

Measured on 8 axon-attached TRN2 NeuronCores: relative error vs the
fp32 jax reference 6.4e-4 (float32r matmuls are ~tf32); device time
~572 us/invocation (For_i-loop wall-clock delta; the cost model
predicts 276 us with ACT exp as the largest single engine load).
"""

import sys

sys.path.insert(0, "/opt/trn_rl_repo")

from contextlib import ExitStack

import numpy as np

import concourse.bass as bass
import concourse.tile as tile
from concourse import bacc, mybir
from concourse.bass_utils import run_bass_kernel_spmd

F32 = mybir.dt.float32
F32R = mybir.dt.float32r

B, D, H, DK = 4, 512, 8, 64
SQ, SK = 2048, 2048
QC = SQ // 2          # per-core query columns
CB = D // 128         # channel blocks of 128
KB = SK // 128        # key blocks of 128
NCH = 512             # matmul moving-dim chunk
SCALE = DK ** -0.5
EPS = 1e-12

# Feature flags (validated in CoreSim; HW verifier confirms at compile)
BOTH_PSUM_TT = False  # HW verifier: only one TT input may be PSUM


def emit_core_kernel(ctx: ExitStack, tc: tile.TileContext, ins: dict, out_ap: bass.AP):
    nc = tc.nc
    EXP = mybir.ActivationFunctionType.Exp
    SQRT = mybir.ActivationFunctionType.Sqrt
    ADD = mybir.AluOpType.add
    MULT = mybir.AluOpType.mult
    MAX = mybir.AluOpType.max
    SUB = mybir.AluOpType.subtract

    p_const = ctx.enter_context(tc.tile_pool(name="const", bufs=1))
    p_persist = ctx.enter_context(tc.tile_pool(name="persist", bufs=1))
    # PSUM: 8 banks total. big = [128,1024] (2 banks) x2, av = [65,1024] x2.
    ps_big = ctx.enter_context(tc.tile_pool(name="psbig", bufs=2, space="PSUM"))
    ps_av = ctx.enter_context(tc.tile_pool(name="psav", bufs=4, space="PSUM"))

    # ---- constants ----
    woT = [p_const.tile([128, D], F32R, tag=f"woT{ci}", name=f"woT{ci}") for ci in range(CB)]
    for ci in range(CB):
        nc.sync.dma_start(out=woT[ci], in_=ins["WoT"][ci * 128:(ci + 1) * 128, :])

    def load_col_vec(name):
        # [512] dram -> [128, CB] sbuf, channel c at (partition c%128, col c//128)
        t = p_const.tile([128, CB], F32, tag=name)
        nc.sync.dma_start(out=t, in_=ins[name].rearrange("(m p) -> p m", p=128))
        return t

    bq_pp = load_col_vec("bq")
    bk_pp = load_col_vec("bk")
    bo_pp = load_col_vec("bo")

    bv_bc = p_const.tile([128, D], F32, tag="bv_bc", name="bv_bc")
    bv_in = ins["bv"]
    nc.sync.dma_start(
        out=bv_bc,
        in_=bass.AP(tensor=bv_in.tensor, offset=bv_in.offset,
                    ap=[[0, 128]] + bv_in.ap),
    )

    def load_gb(gname, bname, tag):
        t = p_const.tile([2, D], F32R, tag=tag)
        nc.sync.dma_start(out=t[0:1, :], in_=ins[gname][None, :])
        nc.sync.dma_start(out=t[1:2, :], in_=ins[bname][None, :])
        return t

    gb0 = load_gb("g0", "b0", "gb0")
    gb1 = load_gb("g1", "b1", "gb1")

    ones_in = ins["ones_c"]  # [128, 8] of ones, f32r
    ones_col = p_const.tile([128, 1], F32R, tag="ones_col", name="ones_col")
    nc.sync.dma_start(out=ones_col, in_=ones_in[:, 0:1])
    onesT = p_const.tile([65, 64], F32R, tag="onesT", name="onesT")
    nc.sync.dma_start(out=onesT[64:65, :], in_=ins["ones_q"][None, 0:64])
    eps_t = p_const.tile([1, 1], F32, tag="eps", name="eps")
    nc.vector.memset(eps_t, EPS)

    # ---- phase 1: projections ----
    qch = [p_persist.tile([128, QC], F32R, tag=f"qch{m}", name=f"qch{m}") for m in range(CB)]
    kch = [p_persist.tile([128, SK], F32R, tag=f"kch{m}", name=f"kch{m}") for m in range(CB)]
    vs = [p_persist.tile([128, H, DK + 1], F32R, tag=f"vs{sb}", name=f"vs{sb}") for sb in range(KB)]

    with tc.tile_pool(name="stage", bufs=1) as p_stage:
        wqT = [p_stage.tile([128, D], F32R, tag=f"wqT{ci}", name=f"wqT{ci}") for ci in range(CB)]
        wkT = [p_stage.tile([128, D], F32R, tag=f"wkT{ci}", name=f"wkT{ci}") for ci in range(CB)]
        wvT = [p_stage.tile([128, D], F32R, tag=f"wvT{ci}", name=f"wvT{ci}") for ci in range(CB)]
        qc = [p_stage.tile([128, QC], F32R, tag=f"qc{ci}", name=f"qc{ci}") for ci in range(CB)]
        kc = [p_stage.tile([128, SK], F32R, tag=f"kc{ci}", name=f"kc{ci}") for ci in range(CB)]
        for ci in range(CB):
            sl = slice(ci * 128, (ci + 1) * 128)
            nc.sync.dma_start(out=wqT[ci], in_=ins["WqT"][sl, :])
            nc.sync.dma_start(out=wkT[ci], in_=ins["WkT"][sl, :])
            nc.sync.dma_start(out=wvT[ci], in_=ins["WvT"][sl, :])
            nc.sync.dma_start(out=qc[ci], in_=ins["Qc"][sl, :])
            nc.sync.dma_start(out=kc[ci], in_=ins["Kc"][sl, :])

        # q projection: qch[m] = sum_ci WqT[ci]^T-block @ Qc[ci] + bq
        for m in range(CB):
            mcols = slice(m * 128, (m + 1) * 128)
            ps = ps_big.tile([128, QC], F32, tag="big", name="big")
            for n0 in range(0, QC, NCH):
                for ci in range(CB):
                    nc.tensor.matmul(
                        out=ps[:, n0:n0 + NCH],
                        lhsT=(wqT[ci][:, mcols]),
                        rhs=(qc[ci][:, n0:n0 + NCH]),
                        start=(ci == 0), stop=(ci == CB - 1),
                    )
            nc.vector.tensor_scalar(
                out=qch[m], in0=ps, scalar1=bq_pp[:, m:m + 1], scalar2=None, op0=ADD)

        # k projection (channel-major)
        for m in range(CB):
            mcols = slice(m * 128, (m + 1) * 128)
            for n0 in range(0, SK, NCH):
                ps = ps_av.tile([128, NCH], F32, tag="av", name="av")
                for ci in range(CB):
                    nc.tensor.matmul(
                        out=ps,
                        lhsT=(wkT[ci][:, mcols]),
                        rhs=(kc[ci][:, n0:n0 + NCH]),
                        start=(ci == 0), stop=(ci == CB - 1),
                    )
                nc.vector.tensor_scalar(
                    out=kch[m][:, n0:n0 + NCH], in0=ps,
                    scalar1=bk_pp[:, m:m + 1], scalar2=None, op0=ADD)

        # v projection (sequence-major, + ones column for denominators)
        for sb in range(KB):
            ps = ps_av.tile([128, NCH], F32, tag="av", name="av")
            for ci in range(CB):
                nc.tensor.matmul(
                    out=ps,
                    lhsT=(kc[ci][:, sb * 128:(sb + 1) * 128]),
                    rhs=(wvT[ci][:, 0:D]),
                    start=(ci == 0), stop=(ci == CB - 1),
                )
            nc.vector.tensor_add(
                vs[sb][:, :, 0:DK],
                ps.rearrange("p (h d) -> p h d", h=H),
                bv_bc.rearrange("p (h d) -> p h d", h=H),
            )
            nc.sync.dma_start(out=vs[sb][:, :, DK:DK + 1],
                              in_=ins["ones_c"][:, :, None])

    # ---- phase 2: attention ----
    # (pools created after the staging pool released its SBUF)
    p_exp = ctx.enter_context(tc.tile_pool(name="exp", bufs=4))
    p_att = ctx.enter_context(tc.tile_pool(name="att", bufs=2))
    p_rec = ctx.enter_context(tc.tile_pool(name="rec", bufs=2))
    p_xz = ctx.enter_context(tc.tile_pool(name="xz", bufs=5))
    p_work = ctx.enter_context(tc.tile_pool(name="work", bufs=4))
    p_tmp = ctx.enter_context(tc.tile_pool(name="tmp", bufs=2))
    p_small = ctx.enter_context(tc.tile_pool(name="small", bufs=4))
    x = [None] * CB  # channel-major q+att blocks
    for hp in range(H // 2):  # head pairs sharing a channel block
        m = hp
        att_n = {}
        for par, h in ((0, 2 * hp), (64, 2 * hp + 1)):
            hsl = slice(par, par + DK)
            avc = [ps_av.tile([DK + 1, NCH], F32, tag="av", name="avc")
                   for _ in range(QC // NCH)]
            for kb in range(KB):
                lps = ps_big.tile([128, QC], F32, tag="big", name="big")
                for n0 in range(0, QC, NCH):
                    nc.tensor.matmul(
                        out=lps[:, n0:n0 + NCH],
                        lhsT=(kch[m][hsl, kb * 128:(kb + 1) * 128]),
                        rhs=(qch[m][hsl, n0:n0 + NCH]),
                        start=True, stop=True,
                    )
                et = p_exp.tile([128, QC], F32R, tag="exp", name="exp")
                nc.scalar.activation(et, lps, EXP, bias=0.0, scale=SCALE)
                for ci, n0 in enumerate(range(0, QC, NCH)):
                    nc.tensor.matmul(
                        out=avc[ci],
                        lhsT=(vs[kb][:, h, :]),
                        rhs=(et[:, n0:n0 + NCH]),
                        start=(kb == 0), stop=(kb == KB - 1),
                    )
            # denominator -> reciprocal -> broadcast -> normalized attention
            recs = p_rec.tile([DK + 1, QC], F32R, tag="recs", name="recs")
            for ci, n0 in enumerate(range(0, QC, NCH)):
                nc.vector.tensor_copy(recs[DK:DK + 1, n0:n0 + NCH],
                                      avc[ci][DK:DK + 1, :])
            nc.vector.reciprocal(recs[DK:DK + 1, :], recs[DK:DK + 1, :])
            a_t = p_att.tile([DK, QC], F32R, tag="attn", name="attn")
            for ci, n0 in enumerate(range(0, QC, NCH)):
                rbc = ps_av.tile([DK, NCH], F32, tag="av", name="rbc")
                nc.tensor.matmul(
                    out=rbc,
                    lhsT=(onesT[DK:DK + 1, :]),
                    rhs=(recs[DK:DK + 1, n0:n0 + NCH]),
                    start=True, stop=True,
                )
                rbc_s = p_tmp.tile([DK, NCH], F32R, tag="tmp", name="rbc_s")
                nc.vector.tensor_copy(rbc_s, rbc)
                nc.vector.tensor_mul(a_t[:, n0:n0 + NCH], avc[ci][0:DK, :], rbc_s)
            att_n[par] = a_t

        # assemble x[m] = qch[m] + att: even head rows 0:64, odd rows 64:128
        attb = p_att.tile([128, QC], F32R, tag="attb", name="attb", bufs=2)
        nc.sync.dma_start(out=attb[0:DK, :], in_=att_n[0])
        nc.sync.dma_start(out=attb[DK:128, :], in_=att_n[64])
        xm = p_xz.tile([128, QC], F32R, tag="xz", name="xz")
        nc.gpsimd.tensor_add(xm, attb, qch[m])
        x[m] = xm

    # ---- phase 3: tail (LN0 -> conv+relu residual -> LN1) ----
    def layernorm(blocks, gb, out_pool, out_tag):
        # channel-axis layernorm on 4x [128, QC] blocks
        sum_ps = [ps_av.tile([1, NCH], F32, tag="av", name="sum_ps")
                  for _ in range(QC // NCH)]
        sq_ps = [ps_av.tile([1, NCH], F32, tag="av", name="sq_ps")
                 for _ in range(QC // NCH)]
        for m in range(CB):
            for ci, n0 in enumerate(range(0, QC, NCH)):
                nc.tensor.matmul(
                    out=sum_ps[ci],
                    lhsT=(ones_col),
                    rhs=(blocks[m][:, n0:n0 + NCH]),
                    start=(m == 0), stop=(m == CB - 1),
                )
        for m in range(CB):
            sq_t = p_tmp.tile([128, QC], F32R, tag="tmp", name="sq")
            nc.gpsimd.tensor_mul(sq_t, blocks[m], blocks[m])
            for ci, n0 in enumerate(range(0, QC, NCH)):
                nc.tensor.matmul(
                    out=sq_ps[ci],
                    lhsT=(ones_col),
                    rhs=(sq_t[:, n0:n0 + NCH]),
                    start=(m == 0), stop=(m == CB - 1),
                )

        mean = p_small.tile([1, QC], F32, tag="stat", name="mean")
        ex2 = p_small.tile([1, QC], F32, tag="stat", name="ex2")
        for ci, n0 in enumerate(range(0, QC, NCH)):
            nc.vector.tensor_scalar(
                out=mean[:, n0:n0 + NCH], in0=sum_ps[ci],
                scalar1=1.0 / D, scalar2=None, op0=MULT)
            nc.vector.tensor_scalar(
                out=ex2[:, n0:n0 + NCH], in0=sq_ps[ci],
                scalar1=1.0 / D, scalar2=None, op0=MULT)
        var = p_small.tile([1, QC], F32, tag="stat", name="var")
        nc.vector.tensor_mul(var, mean, mean)
        nc.vector.tensor_sub(var, ex2, var)
        sd = p_small.tile([1, QC], F32, tag="stat", name="sd")
        nc.scalar.activation(sd, var, SQRT, bias=eps_t, scale=1.0)
        rstd = p_small.tile([1, QC], F32R, tag="stat", name="rstd")
        nc.vector.reciprocal(rstd, sd)
        # rhsB rows: [0] = -mean*rstd, [1] = ones
        rhsB = p_small.tile([2, QC], F32R, tag="stat", name="rhsB")
        nc.sync.dma_start(out=rhsB[1:2, :], in_=ins["ones_q"][None, :])
        mr = p_small.tile([1, QC], F32, tag="stat", name="mr")
        nc.vector.tensor_mul(mr, mean, rstd)
        nc.vector.tensor_scalar(
            out=rhsB[0:1, :], in0=mr, scalar1=-1.0, scalar2=None, op0=MULT)

        outs = []
        for m in range(CB):
            mcols = slice(m * 128, (m + 1) * 128)
            a_ps = ps_big.tile([128, QC], F32, tag="big", name="big")
            b_ps = ps_big.tile([128, QC], F32, tag="big", name="big")
            for n0 in range(0, QC, NCH):
                nc.tensor.matmul(
                    out=a_ps[:, n0:n0 + NCH],
                    lhsT=(gb[0:1, mcols]),
                    rhs=(rstd[:, n0:n0 + NCH]),
                    start=True, stop=True,
                )
                nc.tensor.matmul(
                    out=b_ps[:, n0:n0 + NCH],
                    lhsT=(gb[0:2, mcols]),
                    rhs=(rhsB[:, n0:n0 + NCH]),
                    start=True, stop=True,
                )
            t = p_tmp.tile([128, QC], F32R, tag="tmp", name="lnt")
            nc.vector.tensor_mul(t, blocks[m], a_ps)
            o = out_pool.tile([128, QC], F32R, tag=out_tag)
            nc.vector.tensor_add(o, t, b_ps)
            outs.append(o)
        return outs

    y0 = layernorm(x, gb0, p_work, "work")

    z = []
    for o in range(CB):
        ocols = slice(o * 128, (o + 1) * 128)
        cps = ps_big.tile([128, QC], F32, tag="big", name="big")
        for n0 in range(0, QC, NCH):
            for ci in range(CB):
                nc.tensor.matmul(
                    out=cps[:, n0:n0 + NCH],
                    lhsT=(woT[ci][:, ocols]),
                    rhs=(y0[ci][:, n0:n0 + NCH]),
                    start=(ci == 0), stop=(ci == CB - 1),
                )
        r_t = p_tmp.tile([128, QC], F32R, tag="tmp", name="relu")
        nc.scalar.activation(r_t, cps, mybir.ActivationFunctionType.Relu,
                             bias=bo_pp[:, o:o + 1], scale=1.0)
        zo = p_xz.tile([128, QC], F32R, tag="xz", name="xz")
        nc.gpsimd.tensor_add(zo, r_t, y0[o])
        z.append(zo)

    fin = layernorm(z, gb1, p_xz, "xz")
    for m in range(CB):
        nc.sync.dma_start(out=out_ap[m * 128:(m + 1) * 128, :], in_=fin[m])


def build_module():
    nc = bacc.Bacc("TRN2", target_bir_lowering=False, debug=False)
    ins = {}
    ins["Qc"] = nc.dram_tensor("Qc", [D, QC], F32R, kind="ExternalInput").ap()
    ins["Kc"] = nc.dram_tensor("Kc", [D, SK], F32R, kind="ExternalInput").ap()
    for w in ("WqT", "WkT", "WvT", "WoT"):
        ins[w] = nc.dram_tensor(w, [D, D], F32R, kind="ExternalInput").ap()
    for vname in ("bq", "bk", "bv", "bo"):
        ins[vname] = nc.dram_tensor(vname, [D], F32, kind="ExternalInput").ap()
    for vname in ("g0", "b0", "g1", "b1"):
        ins[vname] = nc.dram_tensor(vname, [D], F32R, kind="ExternalInput").ap()
    ins["ones_c"] = nc.dram_tensor("ones_c", [128, H], F32R,
                                   kind="ExternalInput").ap()
    ins["ones_q"] = nc.dram_tensor("ones_q", [QC], F32R,
                                   kind="ExternalInput").ap()
    out_ap = nc.dram_tensor("out", [D, QC], F32R, kind="ExternalOutput").ap()

    with tile.TileContext(nc) as tc:
        with nc.allow_low_precision(reason="float32r tiles feed full-rate matmuls"):
            with ExitStack() as ctx:
                emit_core_kernel(ctx, tc, ins, out_ap)
    nc.compile()
    return nc


_NC_CACHE = None


def _get_nc():
    global _NC_CACHE
    if _NC_CACHE is None:
        _NC_CACHE = build_module()
    return _NC_CACHE


def kernel(Q, K, Wq, bq, Wk, bk, Wv, bv, Wo, bo, gamma0, beta0, gamma1, beta1,
           _trace=False, _trace_cores=None):
    Q = np.ascontiguousarray(np.asarray(Q, dtype=np.float32))
    K = np.ascontiguousarray(np.asarray(K, dtype=np.float32))
    shared = {
        "WqT": np.ascontiguousarray(np.asarray(Wq, np.float32).T),
        "WkT": np.ascontiguousarray(np.asarray(Wk, np.float32).T),
        "WvT": np.ascontiguousarray(np.asarray(Wv, np.float32).T),
        "WoT": np.ascontiguousarray(np.asarray(Wo, np.float32).T),
        "bq": np.asarray(bq, np.float32), "bk": np.asarray(bk, np.float32),
        "bv": np.asarray(bv, np.float32), "bo": np.asarray(bo, np.float32),
        "g0": np.asarray(gamma0, np.float32), "b0": np.asarray(beta0, np.float32),
        "g1": np.asarray(gamma1, np.float32), "b1": np.asarray(beta1, np.float32),
        "ones_c": np.ones((128, H), dtype=np.float32),
        "ones_q": np.ones((QC,), dtype=np.float32),
    }
    in_maps = []
    for core in range(8):
        b, j = core // 2, core % 2
        m = dict(shared)
        m["Qc"] = np.ascontiguousarray(Q[b, :, j * QC:(j + 1) * QC])
        m["Kc"] = np.ascontiguousarray(K[b])
        in_maps.append(m)

    nc = _get_nc()
    res = run_bass_kernel_spmd(
        nc, in_maps, core_ids=list(range(8)),
        trace=_trace, trace_cores=_trace_cores,
    )
    out = np.empty((B, D, SQ), dtype=np.float32)
    for core in range(8):
        b, j = core // 2, core % 2
        out[b, :, j * QC:(j + 1) * QC] = res.results[core]["out"]
    if _trace:
        kernel._last_result = res
    return out
